# revision 14
# baseline (speedup 1.0000x reference)
"""Trainium2 Bass kernel for nn_Attention_Critic (8-agent attention critic).

Data-parallel over batch across 8 NeuronCores (2048 batch rows per core).
BatchNorm statistics are computed on host in f32 over the full batch and
folded into the encoder weights, so the device program has no collective.

Device layout: feature-on-partitions, [feat, batch] tiles.
  - host pre-transposes s|a to [agent, 80, 2048] per core (bf16)
  - encoders / projections: PE matmuls, contraction on partitions
  - attention einsum pibd,pjbd->pijb: DVE pairwise muls on [128=(head,d), B]
    tiles + PE block-ones matmuls reducing over d (partition reduce) into a
    compact [32=(4j+p), B] logits block
  - softmax: ACT exp, PE ones-matmul for the j-sum, DVE reciprocal; weights
    broadcast d-wise back to [128, B] by a second ones-pattern matmul
  - weighted sum over agents folded into the critic's PSUM accumulation
  - q-gather: host-computed argmax index (f32) -> one-hot via is_equal

All weights/constants are packed into two blobs (one bf16, one f32) so a
call stages only 3 host arrays; the runner keeps the compiled executable
and device-resident inputs cached across calls.
"""

import hashlib
import sys

sys.path.insert(0, "/opt/trn_rl_repo")

import numpy as np
from ml_dtypes import bfloat16

from concourse import bacc, bass, mybir, tile
from concourse.bass_utils import run_bass_kernel_spmd

F32 = mybir.dt.float32
BF16 = mybir.dt.bfloat16
AF = mybir.ActivationFunctionType
ALU = mybir.AluOpType
AX = mybir.AxisListType

N_AGENTS = 8
BATCH = 16384
SDIM, ADIM = 64, 16
CDIM = SDIM + ADIM  # 80
HID = 128
HEADS = 4
ATT_D = 32
N_CORES = 8
SH = BATCH // N_CORES  # 2048 batch rows per core
NH = 2  # batch halves per core
BH = SH // NH  # 1024
NC512 = BH // 512  # matmul chunks per half
EPS = 1e-5
ISQD = float(1.0 / np.sqrt(np.float32(ATT_D)))
SLOPE = 0.01  # LeakyReLU negative slope

# ---- packed bf16 weight blob column offsets (width-128 stripes) ----
O_WSA = 0                     # 8 x [80, 128]
O_WSE = O_WSA + 8 * 128       # 8 x [64, 128]
O_WK = O_WSE + 8 * 128        # [128, 128]
O_WSL = O_WK + 128
O_WV = O_WSL + 128
O_WC1A = O_WV + 128           # 8 x [128, 128]
O_WC1B = O_WC1A + 8 * 128     # 8 x [128, 128]
O_WC2 = O_WC1B + 8 * 128      # [128, 8*16]
O_CBC = O_WC2 + 128           # 4 x [128, 128]
O_CRED = O_CBC + 4 * 128      # [128, 32]
O_CZ = O_CRED + 32            # [128, 4]
WB_TOT = O_CZ + 4             # 5156

# ---- packed f32 blob column offsets ----
F_BSA = 0                     # 8 x [128, 1]
F_BSE = F_BSA + 8
F_BC1 = F_BSE + 8
F_BV = F_BC1 + 8              # [128, 1]
F_BC2B = F_BV + 1             # 8 x [128, 16]
F_CREP = F_BC2B + 8 * 16      # [4, 128]
F_IOTA = F_CREP + 128         # [128, 8*16] : [p, 16t+k] = k
F_IDX = F_IOTA + 128          # per-core [p, 16i+8h+t]
FB_TOT = F_IDX + 128          # 537

_CACHE = {}


def _build():
    nc = bacc.Bacc(None, num_devices=N_CORES)

    xt = nc.declare_dram_parameter("xt", [N_AGENTS, CDIM, SH], BF16, isOutput=False)
    wb = nc.declare_dram_parameter("wb", [128, WB_TOT], BF16, isOutput=False)
    fb = nc.declare_dram_parameter("fb", [128, FB_TOT], F32, isOutput=False)
    # every core outputs the FULL gathered result (replicated), so the host
    # fetch reads a single shard from one device instead of 8
    out = nc.declare_dram_parameter(
        "out", [N_CORES * N_AGENTS, SH], BF16, isOutput=True)
    outl = nc.dram_tensor("outl", [N_AGENTS, SH], BF16)
    outg = nc.dram_tensor(
        "outg", [N_CORES * N_AGENTS, SH], BF16, addr_space="Shared")

    with tile.TileContext(nc) as tc:
        with (
            tc.tile_pool(name="wpool", bufs=1) as wp,     # resident weight blobs
            tc.tile_pool(name="xpool", bufs=1) as xp,     # xt tiles
            tc.tile_pool(name="work", bufs=3) as wkp,     # big transient tiles
            tc.tile_pool(name="kvpool", bufs=1) as kvp,   # keys/sel/vals/se per half
            tc.tile_pool(name="attn", bufs=2) as atp,     # exp/w tiles
            tc.tile_pool(name="qp", bufs=4) as qp,        # q path tiles
            tc.tile_pool(name="ps", bufs=3, space="PSUM") as ps,    # [128,1024]
            tc.tile_pool(name="wbps", bufs=2, space="PSUM") as wbp,  # [128,512]
        ):
            # ---------- resident weight/const blobs (2 DMAs) ----------
            wbt = wp.tile([128, WB_TOT], BF16, tag="wbt", name="wbt")
            nc.sync.dma_start(wbt[:], wb[:])
            fbt = wp.tile([128, FB_TOT], F32, tag="fbt", name="fbt")
            nc.sync.dma_start(fbt[:], fb[:])

            w_sa2 = [wbt[0:CDIM, O_WSA + 128 * n:O_WSA + 128 * (n + 1)]
                     for n in range(N_AGENTS)]
            w_se2 = [wbt[0:SDIM, O_WSE + 128 * n:O_WSE + 128 * (n + 1)]
                     for n in range(N_AGENTS)]
            w_k = wbt[:, O_WK:O_WK + 128]
            w_sl = wbt[:, O_WSL:O_WSL + 128]
            w_v = wbt[:, O_WV:O_WV + 128]
            w_c1a = [wbt[:, O_WC1A + 128 * n:O_WC1A + 128 * (n + 1)]
                     for n in range(N_AGENTS)]
            w_c1b = [wbt[:, O_WC1B + 128 * n:O_WC1B + 128 * (n + 1)]
                     for n in range(N_AGENTS)]
            w_c2 = [wbt[:, O_WC2 + ADIM * n:O_WC2 + ADIM * (n + 1)]
                    for n in range(N_AGENTS)]
            st_bc = [wbt[:, O_CBC + 128 * j:O_CBC + 128 * (j + 1)] for j in range(4)]
            ones_red = wbt[:, O_CRED:O_CRED + 32]
            ones_z = wbt[:, O_CZ:O_CZ + 4]
            b_sa2 = [fbt[:, F_BSA + n:F_BSA + n + 1] for n in range(N_AGENTS)]
            b_se2 = [fbt[:, F_BSE + n:F_BSE + n + 1] for n in range(N_AGENTS)]
            b_c1 = [fbt[:, F_BC1 + n:F_BC1 + n + 1] for n in range(N_AGENTS)]
            b_v = fbt[:, F_BV:F_BV + 1]
            bc2b = [fbt[:, F_BC2B + ADIM * n:F_BC2B + ADIM * (n + 1)]
                    for n in range(N_AGENTS)]
            ones_rep = fbt[0:HEADS, F_CREP:F_CREP + 128]
            iota_rep = fbt[:, F_IOTA:F_IOTA + 128]
            idx_all = fbt[:, F_IDX:F_IDX + 128]

            # ---------- input tiles ----------
            xts = []
            for n in range(N_AGENTS):
                xt_n = xp.tile([CDIM, SH], BF16, tag=f"xt{n}", name=f"xt{n}")
                nc.sync.dma_start(xt_n[:], xt[n])
                xts.append(xt_n)

            # ---------- phases per batch-half ----------
            for h in range(NH):
                hs = h * BH
                # phase 2: encoders -> keys/sel/vals/se for all agents
                keys, sel, vals, se = [], [], [], []
                for n in range(N_AGENTS):
                    xv = xts[n][:, hs:hs + BH]
                    e_ps = ps.tile([HID, BH], F32, tag="ps", name="ps")
                    for c in range(NC512):
                        cs = slice(512 * c, 512 * (c + 1))
                        nc.tensor.matmul(e_ps[:, cs], w_sa2[n], xv[:, cs],
                                         start=True, stop=True)
                    e_n = wkp.tile([HID, BH], BF16, tag="en", name="en")
                    nc.scalar.activation(e_n[:], e_ps[:], AF.Lrelu, bias=b_sa2[n],
                                         alpha=SLOPE)
                    se_ps = ps.tile([HID, BH], F32, tag="ps", name="ps")
                    for c in range(NC512):
                        cs = slice(512 * c, 512 * (c + 1))
                        nc.tensor.matmul(se_ps[:, cs], w_se2[n],
                                         xv[0:SDIM, cs], start=True, stop=True)
                    se_n = kvp.tile([HID, BH], BF16, tag=f"se{n}", name=f"se{n}")
                    nc.scalar.activation(se_n[:], se_ps[:], AF.Lrelu, bias=b_se2[n],
                                         alpha=SLOPE)
                    se.append(se_n)
                    k_ps = ps.tile([HID, BH], F32, tag="ps", name="ps")
                    for c in range(NC512):
                        cs = slice(512 * c, 512 * (c + 1))
                        nc.tensor.matmul(k_ps[:, cs], w_k, e_n[:, cs],
                                         start=True, stop=True)
                    k_n = kvp.tile([HID, BH], BF16, tag=f"k{n}", name=f"k{n}")
                    nc.scalar.copy(k_n[:], k_ps[:])
                    keys.append(k_n)
                    sl_ps = ps.tile([HID, BH], F32, tag="ps", name="ps")
                    for c in range(NC512):
                        cs = slice(512 * c, 512 * (c + 1))
                        nc.tensor.matmul(sl_ps[:, cs], w_sl, se_n[:, cs],
                                         start=True, stop=True)
                    sl_n = kvp.tile([HID, BH], BF16, tag=f"sl{n}", name=f"sl{n}")
                    nc.scalar.copy(sl_n[:], sl_ps[:])
                    sel.append(sl_n)
                    v_ps = ps.tile([HID, BH], F32, tag="ps", name="ps")
                    for c in range(NC512):
                        cs = slice(512 * c, 512 * (c + 1))
                        nc.tensor.matmul(v_ps[:, cs], w_v, e_n[:, cs],
                                         start=True, stop=True)
                    v_n = kvp.tile([HID, BH], BF16, tag=f"v{n}", name=f"v{n}")
                    nc.scalar.activation(v_n[:], v_ps[:], AF.Lrelu, bias=b_v,
                                         alpha=SLOPE)
                    vals.append(v_n)

                # phases 3+4: attention + critic + q, per agent i
                for i in range(N_AGENTS):
                    jall = [j for j in range(N_AGENTS) if j != i]
                    # --- logits: two [128,BH] psum tiles (j 0-3 | j 4-7), row
                    # block 32*(j%4) holds pair (i,j); diag computed then zeroed
                    lgA = ps.tile([HID, BH], F32, tag="ps", name="lgA")
                    lgB = ps.tile([HID, BH], F32, tag="ps", name="lgB")
                    for j in range(N_AGENTS):
                        if j == i:
                            continue
                        prod = wkp.tile([HID, BH], BF16, tag="prod", name="prod")
                        nc.vector.tensor_tensor(prod[:], sel[i][:], keys[j][:],
                                                ALU.mult)
                        lg = lgA if j < 4 else lgB
                        jj = j % 4
                        for c in range(NC512):
                            cs = slice(512 * c, 512 * (c + 1))
                            nc.tensor.matmul(lg[32 * jj:32 * (jj + 1), cs],
                                             ones_red, prod[:, cs],
                                             start=True, stop=True,
                                             tile_position=(0, 32 * jj))
                    # --- exp (scaled); diagonal row-block zeroed after ---
                    exA = atp.tile([HID, BH], BF16, tag="exA", name="exA")
                    exB = atp.tile([HID, BH], BF16, tag="exB", name="exB")
                    nc.scalar.activation(exA[:], lgA[:], AF.Exp, scale=ISQD)
                    nc.scalar.activation(exB[:], lgB[:], AF.Exp, scale=ISQD)
                    exd = exA if i < 4 else exB
                    nc.vector.memset(exd[32 * (i % 4):32 * (i % 4 + 1), :], 0.0)
                    # --- Z = sum_j exp -> [4, BH]; w = exp / Z ---
                    z_ps = ps.tile([HEADS, BH], F32, tag="ps", name="zps")
                    for c in range(NC512):
                        cs = slice(512 * c, 512 * (c + 1))
                        nc.tensor.matmul(z_ps[:, cs], ones_z, exA[:, cs],
                                         start=True, stop=False)
                        nc.tensor.matmul(z_ps[:, cs], ones_z, exB[:, cs],
                                         start=False, stop=True)
                    rz = atp.tile([HEADS, BH], F32, tag="rz", name="rz")
                    nc.vector.reciprocal(rz[:], z_ps[:])
                    rzr_ps = ps.tile([HID, BH], F32, tag="ps", name="rzrps")
                    for c in range(NC512):
                        cs = slice(512 * c, 512 * (c + 1))
                        nc.tensor.matmul(rzr_ps[:, cs], ones_rep, rz[:, cs],
                                         start=True, stop=True)
                    rzr_sb = atp.tile([HID, BH], BF16, tag="rzrsb", name="rzrsb")
                    nc.scalar.copy(rzr_sb[:], rzr_ps[:])
                    wgA = atp.tile([HID, BH], BF16, tag="wgA", name="wgA")
                    wgB = atp.tile([HID, BH], BF16, tag="wgB", name="wgB")
                    nc.vector.tensor_tensor(wgA[:], exA[:], rzr_sb[:], ALU.mult)
                    nc.vector.tensor_tensor(wgB[:], exB[:], rzr_sb[:], ALU.mult)
                    # --- critic h: Wc1a^T se_i + sum_j Wc1b^T (bcast(w_ij)*v_j) ---
                    h_ps = ps.tile([HID, BH], F32, tag="ps", name="h_ps")
                    for c in range(NC512):
                        cs = slice(512 * c, 512 * (c + 1))
                        nc.tensor.matmul(h_ps[:, cs], w_c1a[i], se[i][:, cs],
                                         start=True, stop=False)
                        for idx, j in enumerate(jall):
                            wsrc = wgA if j < 4 else wgB
                            wb_ps = wbp.tile([HID, 512], F32, tag="wb", name="wb")
                            nc.tensor.matmul(wb_ps[:], st_bc[j % 4],
                                             wsrc[:, cs], start=True, stop=True)
                            wv_t = wkp.tile([HID, 512], BF16, tag="wvt", name="wvt")
                            nc.vector.tensor_tensor(wv_t[:], vals[j][:, cs],
                                                    wb_ps[:], ALU.mult)
                            nc.tensor.matmul(h_ps[:, cs], w_c1b[i], wv_t[:],
                                             start=False, stop=(idx == len(jall) - 1))
                    h_i = wkp.tile([HID, BH], BF16, tag="hi", name="hi")
                    nc.scalar.activation(h_i[:], h_ps[:], AF.Lrelu, bias=b_c1[i],
                                         alpha=SLOPE)
                    # --- all_q natural layout via stationary-activation matmul ---
                    aq_ps = wbp.tile([128, 8 * ADIM], F32, tag="wb", name="aq")
                    for t in range(8):  # 8 b-tiles of 128 in this half
                        nc.tensor.matmul(aq_ps[:, ADIM * t:ADIM * (t + 1)],
                                         h_i[:, 128 * t:128 * (t + 1)], w_c2[i],
                                         start=True, stop=True)
                    aq = qp.tile([128, 8 * ADIM], F32, tag="aqsb", name="aqsb")
                    aq3 = aq[:].rearrange("p (t k) -> p t k", t=8)
                    nc.vector.tensor_tensor(
                        aq3, aq_ps[:].rearrange("p (t k) -> p t k", t=8),
                        bc2b[i].unsqueeze(1).broadcast_to([128, 8, ADIM]),
                        ALU.add)
                    # --- host-computed argmax index -> one-hot gather ---
                    eq = qp.tile([128, 8 * ADIM], F32, tag="eq", name="eq")
                    eq3 = eq[:].rearrange("p (t k) -> p t k", t=8)
                    idx_t = idx_all[:, 16 * i + 8 * h:16 * i + 8 * h + 8]
                    nc.vector.tensor_tensor(
                        eq3, iota_rep.rearrange("p (t k) -> p t k", t=8),
                        idx_t.unsqueeze(2).broadcast_to([128, 8, ADIM]),
                        ALU.is_equal)
                    nc.vector.tensor_tensor(eq3, eq3, aq3, ALU.mult)
                    q_i = qp.tile([128, 8], BF16, tag="qi", name="qi")
                    # one-hot-masked row: at most one nonzero per reduction,
                    # so bf16 accumulation is exact up to the final rounding
                    with nc.allow_low_precision(reason="one-hot gather sum"):
                        nc.vector.tensor_reduce(q_i[:], eq3, AX.X, ALU.add)
                    nc.sync.dma_start(
                        outl[i, hs:hs + BH].rearrange("(t p) -> p t", p=128),
                        q_i[:])

            # gather all cores' 32KB results so the output is replicated
            nc.gpsimd.collective_compute(
                "AllGather", ALU.bypass,
                replica_groups=[list(range(N_CORES))],
                ins=[outl[:]],
                outs=[outg[:]],
            )
            gath = qp.tile([N_CORES * N_AGENTS, SH], BF16, tag="gath",
                           name="gath")
            nc.sync.dma_start(gath[:], outg[:])
            nc.sync.dma_start(out[:], gath[:])

    nc.compile()
    return nc


def _get_nc():
    if "nc" not in _CACHE:
        _CACHE["nc"] = _build()
    return _CACHE["nc"]


def make_in_maps(s, a, W_sa, b_sa, W_se, b_se, Wk, Wsel, Wv, bv, Wc1, bc1, Wc2, bc2):
    s = np.asarray(s, np.float32)
    a = np.asarray(a, np.float32)
    x = np.concatenate([s, a], axis=-1)  # [8, 16384, 80]

    # fold training-mode BatchNorm (global batch stats, f32) into encoders
    m = x.mean(axis=1)  # [8, 80]
    v = x.var(axis=1)   # biased
    rstd = 1.0 / np.sqrt(v + EPS)
    mr = m * rstd
    W_sa = np.asarray(W_sa, np.float32)
    W_se = np.asarray(W_se, np.float32)
    wsa2 = W_sa * rstd[:, :, None]
    bsa2 = np.asarray(b_sa, np.float32) - np.einsum('ni,nih->nh', mr, W_sa)
    wse2 = W_se * rstd[:, :SDIM, None]
    bse2 = np.asarray(b_se, np.float32) - np.einsum('ni,nih->nh', mr[:, :SDIM], W_se)

    # attention reduction / broadcast constant patterns
    ones_red = np.zeros((HID, 32), np.float32)
    ones_z = np.zeros((HID, HEADS), np.float32)
    ones_rep = np.zeros((HEADS, HID), np.float32)
    st_bc = np.zeros((4, HID, HID), np.float32)
    for p in range(HEADS):
        ones_red[32 * p:32 * (p + 1), 8 * p:8 * (p + 1)] = 1.0
        for j in range(4):
            ones_z[32 * j + 8 * p, p] = 1.0
            ones_rep[p, 32 * j + 8 * p:32 * j + 8 * p + 8] = 1.0
            st_bc[j, 32 * j + 8 * p, 32 * p:32 * (p + 1)] = 1.0

    # ---- bf16 blob (shared across cores) ----
    wbm = np.zeros((128, WB_TOT), np.float32)
    for n in range(N_AGENTS):
        wbm[0:CDIM, O_WSA + 128 * n:O_WSA + 128 * (n + 1)] = wsa2[n]
        wbm[0:SDIM, O_WSE + 128 * n:O_WSE + 128 * (n + 1)] = wse2[n]
        wbm[:, O_WC1A + 128 * n:O_WC1A + 128 * (n + 1)] = \
            np.asarray(Wc1, np.float32)[n, :HID, :]
        wbm[:, O_WC1B + 128 * n:O_WC1B + 128 * (n + 1)] = \
            np.asarray(Wc1, np.float32)[n, HID:, :]
        wbm[:, O_WC2 + ADIM * n:O_WC2 + ADIM * (n + 1)] = \
            np.asarray(Wc2, np.float32)[n]
    wbm[:, O_WK:O_WK + 128] = \
        np.transpose(np.asarray(Wk, np.float32), (1, 0, 2)).reshape(HID, HID)
    wbm[:, O_WSL:O_WSL + 128] = \
        np.transpose(np.asarray(Wsel, np.float32), (1, 0, 2)).reshape(HID, HID)
    wbm[:, O_WV:O_WV + 128] = \
        np.transpose(np.asarray(Wv, np.float32), (1, 0, 2)).reshape(HID, HID)
    for j in range(4):
        wbm[:, O_CBC + 128 * j:O_CBC + 128 * (j + 1)] = st_bc[j]
    wbm[:, O_CRED:O_CRED + 32] = ones_red
    wbm[:, O_CZ:O_CZ + 4] = ones_z
    wb16 = np.ascontiguousarray(wbm.astype(bfloat16))

    # ---- f32 blob (shared part) ----
    fbs = np.zeros((128, F_IDX), np.float32)
    fbs[:, F_BSA:F_BSA + 8] = bsa2.T
    fbs[:, F_BSE:F_BSE + 8] = bse2.T
    fbs[:, F_BC1:F_BC1 + 8] = np.asarray(bc1, np.float32).T
    fbs[:, F_BV] = np.asarray(bv, np.float32).reshape(HID)
    bc2f = np.asarray(bc2, np.float32)
    for n in range(N_AGENTS):
        fbs[:, F_BC2B + ADIM * n:F_BC2B + ADIM * (n + 1)] = \
            np.tile(bc2f[n], (128, 1))
    fbs[0:HEADS, F_CREP:F_CREP + 128] = ones_rep
    fbs[:, F_IOTA:F_IOTA + 128] = np.tile(
        np.arange(ADIM, dtype=np.float32), (128, 8))

    # per-core argmax index (first max, computed from f32 a like the reference)
    ia = np.argmax(np.asarray(a, np.float32), axis=-1).astype(np.float32)

    in_maps = []
    for c in range(N_CORES):
        sl = slice(c * SH, (c + 1) * SH)
        idxc = np.ascontiguousarray(
            ia[:, sl].reshape(N_AGENTS, NH, 8, 128)
            .transpose(3, 0, 1, 2).reshape(128, 128))
        m_c = {
            "xt": np.ascontiguousarray(
                np.transpose(x[:, sl, :], (0, 2, 1))).astype(bfloat16),
            "wb": wb16,
            "fb": np.ascontiguousarray(np.concatenate([fbs, idxc], axis=1)),
        }
        in_maps.append(m_c)
    return in_maps


def _get_runner():
    """Compile once; return a closure that runs the sharded NEFF warm.

    Caches the jit'd executable (the library's run_bass_via_pjrt re-traces,
    re-lowers and re-loads the NEFF on every call) and keeps staged inputs
    resident on device between calls with identical data, so a warm call
    pays only dispatch + execute + output fetch.
    """
    if "runner" in _CACHE:
        return _CACHE["runner"]

    import jax
    from jax.experimental.shard_map import shard_map
    from jax.sharding import Mesh, NamedSharding, PartitionSpec

    from concourse import bass2jax

    nc = _get_nc()
    bass2jax.install_neuronx_cc_hook()

    partition_name = nc.partition_id_tensor.name if nc.partition_id_tensor else None
    in_names, out_names, out_avals, zero_templates = [], [], [], []
    for alloc in nc.m.functions[0].allocations:
        if not isinstance(alloc, mybir.MemoryLocationSet):
            continue
        name = alloc.memorylocations[0].name
        if alloc.kind == "ExternalInput":
            if name != partition_name:
                in_names.append(name)
        elif alloc.kind == "ExternalOutput":
            out_names.append(name)
            shape = tuple(alloc.tensor_shape)
            dtype = mybir.dt.np(alloc.dtype)
            out_avals.append(jax.core.ShapedArray(shape, dtype))
            zero_templates.append((shape, dtype))
    n_params = len(in_names)
    n_outs = len(out_names)
    in_names_full = in_names + out_names + ([partition_name] if partition_name else [])

    def _body(*args):
        operands = list(args)
        if partition_name is not None:
            operands.append(bass2jax.partition_id_tensor())
        return tuple(
            bass2jax._bass_exec_p.bind(
                *operands,
                out_avals=tuple(out_avals),
                in_names=tuple(in_names_full),
                out_names=tuple(out_names),
                lowering_input_output_aliases=(),
                sim_require_finite=True,
                sim_require_nnan=True,
                nc=nc,
            )
        )

    devices = jax.devices()[:N_CORES]
    assert len(devices) == N_CORES, f"need {N_CORES} devices, have {len(jax.devices())}"
    mesh = Mesh(np.asarray(devices), ("core",))
    # inputs are batch-sharded; outputs (and their donated buffers) are
    # replicated — the NEFF AllGathers the per-core results, so the host
    # fetch touches a single device
    sharded = jax.jit(
        shard_map(
            _body,
            mesh=mesh,
            in_specs=(PartitionSpec("core"),) * n_params
            + (PartitionSpec(),) * n_outs,
            out_specs=(PartitionSpec(),) * n_outs,
            check_rep=False,
        ),
        donate_argnums=tuple(range(n_params, n_params + n_outs)),
        keep_unused=True,
    )
    sharding = NamedSharding(mesh, PartitionSpec("core"))
    staged = {}  # identity + content cache of device-resident inputs

    def _stage(in_maps):
        refs = [m[n] for m in in_maps for n in in_names]
        if staged and all(a is b for a, b in zip(staged["refs"], refs)):
            return staged["dev"]
        concat = [
            np.ascontiguousarray(
                np.concatenate([np.asarray(m[n]) for m in in_maps], axis=0)
            )
            for n in in_names
        ]
        h = hashlib.blake2b(digest_size=16)
        for arr in concat:
            h.update(str((arr.shape, str(arr.dtype))).encode())
            h.update(arr.view(np.uint8).reshape(-1))
        digest = h.digest()
        if staged and staged["digest"] == digest:
            staged["refs"] = refs
            return staged["dev"]
        dev = jax.device_put(concat, [sharding] * n_params)
        jax.block_until_ready(dev)
        staged.clear()
        staged.update(refs=refs, digest=digest, dev=dev)
        return dev

    def run(in_maps):
        dev = _stage(in_maps)
        # donate the previous device-resident outputs as this call's output
        # buffers (the kernel overwrites every element) — skips the zero
        # buffer upload; fall back to host zeros on the first call
        prev = staged.pop("prev", None)
        if prev is None:
            prev = [np.zeros(s, d) for s, d in zero_templates]
        out = sharded(*dev, *prev)
        res = [np.asarray(o) for o in out]
        staged["prev"] = out
        return [
            {
                name: res[i].reshape(N_CORES, N_AGENTS, SH)[c]
                for i, name in enumerate(out_names)
            }
            for c in range(N_CORES)
        ]

    _CACHE["runner"] = run
    return run


def run_warm(in_maps):
    """Run the compiled kernel on pre-built per-core input maps."""
    return _get_runner()(in_maps)


def kernel(**inputs):
    in_maps = make_in_maps(**inputs)
    results = run_warm(in_maps)
    outs = [np.asarray(results[c]["out"], np.float32) for c in range(N_CORES)]
    q = np.concatenate(outs, axis=1)  # [8, 16384]
    return q[..., None].astype(np.float32)


if __name__ == "__main__":
    import reference as R
    inp = {k: np.asarray(v) for k, v in R.setup_inputs().items()}
    got = kernel(**inp)
    print("kernel out", got.shape)
